# revision 4
# baseline (speedup 1.0000x reference)
"""Trainium2 Bass kernel for nn_CustomizedRelaModule (gnn_message_passing).

Math (after folding the deterministic permutation skeleton + adj into the
per-channel input weights):
    W[j, c, h] = adj[j, c] * w_in[c, k(j,c), h]   (0 at j == c)
    z[n, c, :] = data[n, :] @ W[:, c, :] + b_in[c]
    out[n, c]  = sum_h tanh(z)[n, c, h] * (neurons[h, c] * w_out[c, h, 0]) + b_out[c]
    returns (adj, out)

Sharding: channel-parallel — each of the 8 cores computes 32 of the 256
channels for all 4096 samples. data.T is replicated (1MB/core in fp32 x 4
blocks), per-channel weights are sliced per core.

Per-core device kernel: for each 1024-wide block of samples,
  z (128p x 1024) = two fp32r matmuls (K=256 split in 2) into PSUM
  h = tanh(z + b_in) on ScalarE (PSUM -> SBUF)
  out (32p x 1024) += G^T @ h   (gate+w_out folded into a (128,32) stationary)
then + b_out on ScalarE and DMA out.
"""

import sys

if "/opt/trn_rl_repo" not in sys.path:
    sys.path.insert(0, "/opt/trn_rl_repo")

import numpy as np

N, V, NH = 4096, 256, 64
NCORES = 8
CPC = V // NCORES            # 32 channels per core
M_PER_CORE = CPC * NH        # 2048 (c_local, h) columns
MT = M_PER_CORE // 128       # 16 m-tiles (2 channels each)
NBLK = 1024                  # sample columns per block (2 PSUM banks)
NB = N // NBLK               # 4 blocks

_CACHE = {}


def _build_program():
    import concourse.bacc as bacc
    import concourse.tile as tile
    import concourse.mybir as mybir

    f32 = mybir.dt.float32
    f32r = mybir.dt.float32r
    Tanh = mybir.ActivationFunctionType.Tanh

    nc = bacc.Bacc("TRN2", target_bir_lowering=False, debug=False,
                   num_devices=NCORES)

    xT_d = nc.dram_tensor("xT", [V, N], f32, kind="ExternalInput").ap()
    w_d = nc.dram_tensor("w", [V, M_PER_CORE], f32, kind="ExternalInput").ap()
    g_d = nc.dram_tensor("g", [128, MT * CPC], f32, kind="ExternalInput").ap()
    b_d = nc.dram_tensor("b", [128, MT], f32, kind="ExternalInput").ap()
    bo_d = nc.dram_tensor("bo", [CPC, 1], f32, kind="ExternalInput").ap()
    out_d = nc.dram_tensor("out", [CPC, N], f32, kind="ExternalOutput").ap()

    with tile.TileContext(nc) as tc:
        with tc.tile_pool(name="const", bufs=1) as constp, \
             tc.tile_pool(name="stage", bufs=2) as stagep, \
             tc.tile_pool(name="xin", bufs=3) as xp, \
             tc.tile_pool(name="xrp", bufs=4) as xrp, \
             tc.tile_pool(name="zpsum", bufs=2, space="PSUM") as zp, \
             tc.tile_pool(name="redpsum", bufs=1, space="PSUM") as rp, \
             tc.tile_pool(name="hbuf", bufs=3) as hp, \
             tc.tile_pool(name="obuf", bufs=2) as op:

            # weights: DMA fp32, round to fp32r on DVE (one-time)
            w_sb = []
            for kt in range(2):
                s = stagep.tile([128, M_PER_CORE], f32, tag="wstage")
                nc.sync.dma_start(s[:], w_d[kt * 128:(kt + 1) * 128, :])
                t = constp.tile([128, M_PER_CORE], f32r, tag=f"w{kt}")
                nc.vector.tensor_copy(t[:], s[:])
                w_sb.append(t)
            gs = stagep.tile([128, MT * CPC], f32, tag="gstage")
            nc.sync.dma_start(gs[:], g_d[:])
            g_sb = constp.tile([128, MT * CPC], f32r, tag="g")
            nc.vector.tensor_copy(g_sb[:], gs[:])
            b_sb = constp.tile([128, MT], f32, tag="b")
            nc.sync.dma_start(b_sb[:], b_d[:])
            bo_sb = constp.tile([CPC, 1], f32, tag="bo")
            nc.sync.dma_start(bo_sb[:], bo_d[:])

            for blk in range(NB):
                nsl_d = slice(blk * NBLK, (blk + 1) * NBLK)
                xr = []
                for kt in range(2):
                    xs = xp.tile([128, NBLK], f32, tag=f"x{kt}")
                    nc.sync.dma_start(
                        xs[:], xT_d[kt * 128:(kt + 1) * 128, nsl_d])
                    t = xrp.tile([128, NBLK], f32r, tag=f"xr{kt}")
                    nc.vector.tensor_copy(t[:], xs[:])
                    xr.append(t)

                red = rp.tile([CPC, NBLK], f32)
                for mt in range(MT):
                    msl = slice(mt * 128, (mt + 1) * 128)
                    z = zp.tile([128, NBLK], f32)
                    for half in range(NBLK // 512):
                        nsl = slice(half * 512, (half + 1) * 512)
                        nc.tensor.matmul(z[:, nsl],
                                         lhsT=w_sb[0][:, msl],
                                         rhs=xr[0][:, nsl],
                                         start=True, stop=False)
                        nc.tensor.matmul(z[:, nsl],
                                         lhsT=w_sb[1][:, msl],
                                         rhs=xr[1][:, nsl],
                                         start=False, stop=True)
                    h = hp.tile([128, NBLK], f32r)
                    nc.scalar.activation(h[:], z[:], Tanh,
                                         bias=b_sb[:, mt:mt + 1])
                    gsl = slice(mt * CPC, (mt + 1) * CPC)
                    for half in range(NBLK // 512):
                        nsl = slice(half * 512, (half + 1) * 512)
                        nc.tensor.matmul(red[:, nsl],
                                         lhsT=g_sb[:, gsl],
                                         rhs=h[:, nsl],
                                         start=(mt == 0), stop=(mt == MT - 1))

                o = op.tile([CPC, NBLK], f32)
                nc.vector.tensor_scalar_add(o[:], red[:], bo_sb[:, 0:1])
                nc.sync.dma_start(out_d[:, nsl_d], o[:])

    nc.compile()
    return nc


def _get_program():
    if "nc" not in _CACHE:
        _CACHE["nc"] = _build_program()
    return _CACHE["nc"]


def _make_in_maps(data, adj, w_in, b_in, w_out, neurons, b_out):
    xT = np.ascontiguousarray(data.T)  # (V, N)

    # Fold adj + permutation skeleton into dense per-channel weights.
    W = np.zeros((V, V, NH), dtype=np.float32)  # (j, c, h)
    for c in range(V):
        ac = adj[:, c]
        W[:c, c, :] = w_in[c, :c, :] * ac[:c, None]
        W[c + 1:, c, :] = w_in[c, c:, :] * ac[c + 1:, None]

    gate = neurons.T * w_out[:, :, 0]  # (V, NH)

    in_maps = []
    for k in range(NCORES):
        cs = slice(CPC * k, CPC * (k + 1))
        Wk = np.ascontiguousarray(W[:, cs, :].reshape(V, M_PER_CORE))
        gk = gate[cs]     # (32, 64)
        bik = b_in[cs]    # (32, 64)
        Gk = np.zeros((128, MT * CPC), dtype=np.float32)
        bk = np.zeros((128, MT), dtype=np.float32)
        for mt in range(MT):
            for i in range(2):
                cl = 2 * mt + i
                Gk[i * NH:(i + 1) * NH, mt * CPC + cl] = gk[cl]
                bk[i * NH:(i + 1) * NH, mt] = bik[cl]
        bok = np.ascontiguousarray(b_out[cs].reshape(CPC, 1).astype(np.float32))
        in_maps.append({"xT": xT, "w": Wk, "g": Gk, "b": bk, "bo": bok})
    return in_maps


def _run(in_maps, trace=False, tmpdir=None):
    from concourse.bass_utils import run_bass_kernel_spmd
    nc = _get_program()
    return run_bass_kernel_spmd(nc, in_maps, core_ids=list(range(NCORES)),
                                trace=trace, tmpdir=tmpdir)


def kernel(data, adj, neurons, w_in, b_in, w_out, b_out, perm):
    data = np.asarray(data, dtype=np.float32)
    adj = np.asarray(adj, dtype=np.float32)
    neurons = np.asarray(neurons, dtype=np.float32)
    w_in = np.asarray(w_in, dtype=np.float32)
    b_in = np.asarray(b_in, dtype=np.float32)
    w_out = np.asarray(w_out, dtype=np.float32)
    b_out = np.asarray(b_out, dtype=np.float32)
    assert data.shape == (N, V)

    in_maps = _make_in_maps(data, adj, w_in, b_in, w_out, neurons, b_out)
    res = _run(in_maps)

    out = np.empty((N, V), dtype=np.float32)
    for k in range(NCORES):
        out[:, CPC * k:CPC * (k + 1)] = res.results[k]["out"].T
    return adj, out
